# revision 14
# baseline (speedup 1.0000x reference)
"""Trainium2 Bass kernel for nn_DirectedSpatialConv (gnn_message_passing).

out[b,o,n,t] = sum_k W[k] @_c ( Abar_k @_j x[b] + (Pe_k+Ce_k) @_e x_other[b] ) + bias

Sharding: data-parallel over batch B=8, one batch element per NeuronCore.
Per core, fully fused 3-phase pipeline over T-slabs of 128:
  stage1 (PE):  agg[k,c][n, t] accumulated in PSUM from Abar^T / Qe^T matmuls
  transpose(PE): agg [n, (k,c)@t] -> [(k,c), n] per t  (128x128 PE transposes)
  stage2 (PE):  out[o, (n,t)] = Wpack^T @ aggT, k-pairs PSUM-accumulated,
                col-groups for the two n-halves; bias fused into the psum copy.
"""

import sys

sys.path.insert(0, "/opt/trn_rl_repo")

import os
import numpy as np
import ml_dtypes
from contextlib import ExitStack

import concourse.bass as bass
import concourse.tile as tile
from concourse import bacc, mybir
from concourse.bass_utils import run_bass_kernel_spmd

# Problem shapes (hardcoded per contest contract).
B, C, N, E, T, K = 8, 64, 128, 256, 1024, 4
O = 64  # output channels
EPS = 1e-3
N_CORES = 8

TT = 128          # t-slab size
NSLAB = T // TT   # 8
TQ = 32           # t-quarter within a slab
BF16 = mybir.dt.bfloat16
F32 = mybir.dt.float32

_compiled = None  # (nc) cache across kernel() calls


def _build(repeat=1):
    nc = bacc.Bacc("TRN2", target_bir_lowering=False, debug=False,
                   num_devices=N_CORES)

    x_d = nc.dram_tensor("x", [C, N, T], F32, kind="ExternalInput").ap()
    xo_d = nc.dram_tensor("xo", [C, E, T], F32, kind="ExternalInput").ap()
    abar_d = nc.dram_tensor("abar", [N, K * N], BF16, kind="ExternalInput").ap()
    qet_d = nc.dram_tensor("qet", [N, 2 * K * N], BF16, kind="ExternalInput").ap()
    wp_d = nc.dram_tensor("wp", [128, 2 * O], BF16, kind="ExternalInput").ap()
    id_d = nc.dram_tensor("ident", [128, 128], BF16, kind="ExternalInput").ap()
    bias_d = nc.dram_tensor("biasv", [128, 1], F32, kind="ExternalInput").ap()
    out_d = nc.dram_tensor("out", [2, O, 64, T], F32, kind="ExternalOutput").ap()

    def _env(k, d):
        return int(os.environ.get(k, d))

    with tile.TileContext(nc) as tc, ExitStack() as ctx:
        consts = ctx.enter_context(tc.tile_pool(name="consts", bufs=1))
        px = ctx.enter_context(tc.tile_pool(name="px", bufs=_env("KB_PX", 2)))
        pxo = ctx.enter_context(tc.tile_pool(name="pxo", bufs=_env("KB_PXO", 2)))
        pagg = ctx.enter_context(tc.tile_pool(name="pagg", bufs=_env("KB_PAGG", 2)))
        pa2 = ctx.enter_context(tc.tile_pool(name="pa2", bufs=_env("KB_PA2", 2)))
        pout = ctx.enter_context(tc.tile_pool(name="pout", bufs=2))
        ps1 = ctx.enter_context(tc.tile_pool(name="ps1", bufs=_env("KB_PS1", 4), space="PSUM"))
        pst = ctx.enter_context(tc.tile_pool(name="pst", bufs=_env("KB_PST", 2), space="PSUM"))
        ps2 = ctx.enter_context(tc.tile_pool(name="ps2", bufs=_env("KB_PS2", 2), space="PSUM"))

        # --- constants into SBUF ---
        abar_sb = consts.tile([N, K, N], BF16)          # [j, k, n]
        nc.sync.dma_start(abar_sb[:], abar_d.rearrange("j (k n) -> j k n", k=K))
        qet_sb = consts.tile([N, 2, K, N], BF16)        # [e_loc, h, k, n]
        nc.sync.dma_start(qet_sb[:], qet_d.rearrange("e (h k n) -> e h k n", h=2, k=K))
        wp_sb = consts.tile([128, 2, O], BF16)          # [kc, pair, o]
        nc.sync.dma_start(wp_sb[:], wp_d.rearrange("p (r o) -> p r o", r=2))
        id_sb = consts.tile([128, 128], BF16)
        nc.sync.dma_start(id_sb[:], id_d)
        bias_sb = consts.tile([128, 1], F32)
        nc.sync.dma_start(bias_sb[:], bias_d)

        x_tiles = {}
        xo_tiles = {}
        out_tiles = {}

        half_dma = bool(int(os.environ.get("KB_HALF_DMA", "0")))

        def emit_loads(s):
            # c-half-split loads, ch0 halves first, so the first matmuls of a
            # slab unblock after half the slab's bytes have landed
            t0 = s * TT
            xt = px.tile([N, C, TT], BF16, tag="x")
            x_tiles[s] = xt
            xos = []
            for h in range(2):
                xot = pxo.tile([128, C, TT], BF16, tag=f"xo{h}", name=f"xo{h}")
                xo_tiles[(s, h)] = xot
                xos.append(xot)
            for ch in range(2):
                cs = slice(ch * 32, ch * 32 + 32)
                src_cs = slice(0, 32) if half_dma else cs
                nc.gpsimd.dma_start(
                    xt[:, cs, :],
                    x_d[src_cs, :, t0:t0 + TT].rearrange("c j t -> j c t"))
                for h in range(2):
                    nc.gpsimd.dma_start(
                        xos[h][:, cs, :],
                        xo_d[src_cs, h * 128:(h + 1) * 128, t0:t0 + TT].rearrange(
                            "c e t -> e c t"))

        def emit_stage1(s, q):
            """Fill AGG quarter: agg[n, t32, kc256] for t in [s*TT+q*TQ, +TQ)."""
            xt = x_tiles[s]
            xo0 = xo_tiles[(s, 0)]
            xo1 = xo_tiles[(s, 1)]
            agg = pagg.tile([N, TQ, 2 * 128], BF16, tag="agg")
            tq0 = q * TQ
            for ch in range(2):
                for k in range(K):
                    psums = [ps1.tile([128, 16, TQ], F32, tag="s1", name="s1p")
                             for _ in range(2)]
                    n_wi = 1 if abl == "no_qe" else 3
                    for wi in range(n_wi):
                        if wi == 0:
                            w_ap, src = abar_sb[:, k, :], xt
                        elif wi == 1:
                            w_ap, src = qet_sb[:, 0, k, :], xo0
                        else:
                            w_ap, src = qet_sb[:, 1, k, :], xo1
                        for cq in range(2):
                            c0 = ch * 32 + cq * 16
                            nc.tensor.matmul(
                                psums[cq][:],
                                w_ap,
                                src[:, c0:c0 + 16, tq0:tq0 + TQ],
                                start=(wi == 0),
                                stop=(wi == n_wi - 1),
                            )
                    for cq in range(2):
                        c0 = ch * 32 + cq * 16
                        kc0 = k * 64 + c0
                        nc.scalar.activation(
                            agg[:, :, kc0:kc0 + 16],
                            psums[cq].rearrange("p c t -> p t c"),
                            mybir.ActivationFunctionType.Copy,
                        )
            return agg

        TB = 8  # t-bundle size for transpose/stage2

        abl = os.environ.get("KB_ABL", "none")

        def emit_t2_group(s, q, g, agg):
            """Transpose t-bundle g of quarter q; returns a2 tiles (per k-pair)."""
            a2s = []
            npairs = 1 if abl == "half_t" else 2
            for p_i in range(npairs):
                pt = pst.tile([128, TB, 128], BF16, tag="pt")
                for dt_ in range(TB):
                    t_in_q = g * TB + dt_
                    nc.tensor.transpose(
                        pt[:, dt_, :],
                        agg[:, t_in_q, p_i * 128:(p_i + 1) * 128],
                        id_sb,
                    )
                a2 = pa2.tile([128, TB, 128], BF16, tag=f"a2_{p_i}")
                nc.vector.tensor_copy(out=a2[:], in_=pt[:])
                a2s.append(a2)
            if abl == "half_t":
                a2s.append(a2s[0])
            return a2s

        def emit_stage2(s, q, g, a2s, out_half):
            """Channel-mix one t-bundle into OUT tile (with bias)."""
            psum = ps2.tile([128, TB, 64], F32, tag="s2")
            for p_i in range(2):
                for cg in range(2):
                    out_ap = psum[cg * 64:(cg + 1) * 64, :, :]
                    tp = None if cg == 0 else (0, 64)
                    nc.tensor.matmul(
                        out_ap,
                        wp_sb[:, p_i, :],
                        a2s[p_i][:, :, cg * 64:(cg + 1) * 64],
                        start=(p_i == 0),
                        stop=(p_i == 1),
                        tile_position=tp,
                    )
            # t offset within the current t-half buffer
            th = (q * TQ + g * TB) % 64
            nc.scalar.activation(
                out_half[:, :, th:th + TB],
                psum.rearrange("p t n -> p n t"),
                mybir.ActivationFunctionType.Identity,
                bias=bias_sb[:],
            )

        def emit_out_dma(s, half, out_half):
            t0 = s * TT + half * 64
            nc.sync.dma_start(
                out_d[:, :, :, t0:t0 + 64].rearrange("h o n t -> (h o) n t"),
                out_half[:],
            )

        # --- software-pipelined emission: stage1(q) runs one quarter ahead of
        # transpose+stage2(q); within a quarter, stage2 lags transposes by one
        # t4-group so the DVE cast copy is off the PE critical path. ---
        QTOT = NSLAB * 4
        agg_q = {}
        pend = []  # [(s, q, g, a2s), ...] awaiting stage2 (lag 2)
        LAG = _env("KB_LAG", 1)
        rep_ctx = tc.For_i(0, repeat, 1) if repeat > 1 else None
        if rep_ctx is not None:
            ctx.enter_context(rep_ctx)

        def do_t2_quarter(qi):
            nonlocal pend
            s, q = divmod(qi, 4)
            half = q // 2
            if q % 2 == 0:
                out_tiles[(s, half)] = pout.tile([128, 64, 64], F32, tag="out", name="outsb")
            oh = out_tiles[(s, half)]
            agg = agg_q.pop(qi)
            for g in range(TQ // TB):
                a2s = emit_t2_group(s, q, g, agg)
                pend.append((s, q, g, a2s))
                if len(pend) > LAG:
                    ps, pq, pg, pa = pend.pop(0)
                    emit_stage2(ps, pq, pg, pa, out_tiles[(ps, pq // 2)])
            if q % 2 == 1:
                # flush: all groups of this half must land before the DMA
                while pend:
                    ps, pq, pg, pa = pend.pop(0)
                    emit_stage2(ps, pq, pg, pa, out_tiles[(ps, pq // 2)])
                emit_out_dma(s, half, oh)

        for qi in range(QTOT + 1):
            if qi < QTOT:
                s, q = divmod(qi, 4)
                if q == 0:
                    emit_loads(s)
                agg_q[qi] = emit_stage1(s, q)
            if qi >= 1:
                do_t2_quarter(qi - 1)

    nc.compile()
    return nc


def _prep_consts(Av, Pe, Ce, W, bias):
    Av = np.asarray(Av, np.float64)
    dis = 1.0 / np.sqrt(Av.sum(-1) + EPS)           # [K, N]
    abar = dis[:, :, None] * Av * dis[:, None, :]   # [K, n, j]
    abar_t = abar.transpose(0, 2, 1)                # [K, j, n]
    abar_in = np.ascontiguousarray(
        abar_t.transpose(1, 0, 2).reshape(N, K * N)).astype(ml_dtypes.bfloat16)

    Qe = np.asarray(Pe, np.float64) + np.asarray(Ce, np.float64)  # [K, n, e]
    qet = Qe.transpose(2, 0, 1)                      # [e, K, n]
    qet_in = np.concatenate([
        qet[:128].reshape(128, K * N),
        qet[128:].reshape(128, K * N),
    ], axis=1).astype(ml_dtypes.bfloat16)            # [e_loc, h*K*n] with h major
    # layout check: qet_in[e, h*K*N + k*N + n] == Qe[k, n, h*128+e]

    W = np.asarray(W, np.float64)                    # [K, o, c]
    wp_in = np.zeros((128, 2 * O), np.float64)
    for p in range(2):
        for ki in range(2):
            # rows ki*64+c, cols p*O+o  <- W[2p+ki, o, c]
            wp_in[ki * 64:(ki + 1) * 64, p * O:(p + 1) * O] = W[2 * p + ki].T
    wp_in = wp_in.astype(ml_dtypes.bfloat16)

    ident = np.eye(128, dtype=ml_dtypes.bfloat16)
    bsum = np.asarray(bias, np.float64).sum(0)       # [O]
    biasv = np.tile(bsum, 2).reshape(128, 1).astype(np.float32)
    return abar_in, qet_in, wp_in, ident, biasv


def kernel(x, x_other, Av, Pe, Ce, W, bias):
    global _compiled
    if _compiled is None:
        _compiled = _build()
    nc = _compiled

    x = np.asarray(x, np.float32)
    xo = np.asarray(x_other, np.float32)
    abar_in, qet_in, wp_in, ident, biasv = _prep_consts(Av, Pe, Ce, W, bias)

    in_maps = []
    for b in range(N_CORES):
        in_maps.append({
            "x": np.ascontiguousarray(x[b]),
            "xo": np.ascontiguousarray(xo[b]),
            "abar": abar_in,
            "qet": qet_in,
            "wp": wp_in,
            "ident": ident,
            "biasv": biasv,
        })
    res = run_bass_kernel_spmd(nc, in_maps, core_ids=list(range(N_CORES)))
    outs = []
    for b in range(N_CORES):
        o = res.results[b]["out"]                    # [2, O, 64, T]
        outs.append(o.transpose(1, 0, 2, 3).reshape(O, N, T))
    return np.stack(outs).astype(np.float32)
